# revision 2
# baseline (speedup 1.0000x reference)
"""BERT self-attention (B=4, S=2048, H=768, 12 heads x 64) on 8 trn2 cores.

Sharding: core c = batch (c//2) x head-half (c%2, 6 heads each).
Each core computes Q/K/V projections for its 6 heads, attention, and a
partial output projection (its heads' slice of Wo). Host sums the two
partials per batch and adds bo.

On-device layout (per core):
  xT   [768, 2048]  bf16  (DMA-transposed x)
  QT/KT per head-pair [128=2x64, 2048] bf16   (head-dim on partitions)
  V    16 tiles [128 keys, 6 heads x 65] bf16 (col 64 of each head = 1.0)
  scores^T [128 keys, 2x512 q] fp32 PSUM (two heads packed via row tiling)
  exp on ScalarE (scale=1/8, bias=mask column), out bf16
  attn@V -> comb [65, 512] PSUM; row 64 = softmax denominator
  combT_A/B [64, 2048] bf16, scaled by 1/denom
  out-proj: 6 x K=64 chunks -> psum [128, 384] x2 -> out [2048, 768] fp32
"""

import numpy as np
import ml_dtypes

B, S, H = 4, 2048, 768
NH, HS = 12, 64
NHL = 6              # heads per core
NHP = 3              # head pairs per core
HCHUNKS = 6          # 768 / 128 contraction chunks
SKT = 16             # key tiles of 128
SQT = 4              # query tiles of 512
QW = 512             # query tile width
N_CORES = 8

_COMPILED = None


def _build():
    import concourse.bass as bass
    import concourse.mybir as mybir
    import concourse.tile as tile
    from concourse import bacc

    fp32 = mybir.dt.float32
    bf16 = mybir.dt.bfloat16
    AF = mybir.ActivationFunctionType

    nc = bacc.Bacc("TRN2", target_bir_lowering=False, debug=False)

    x_d = nc.dram_tensor("x", [S, H], bf16, kind="ExternalInput").ap()
    wq_d = nc.dram_tensor("wq", [H, NHL * HS], bf16, kind="ExternalInput").ap()
    wk_d = nc.dram_tensor("wk", [H, NHL * HS], bf16, kind="ExternalInput").ap()
    wv_d = nc.dram_tensor("wv", [H, NHL * HS], bf16, kind="ExternalInput").ap()
    wo_d = nc.dram_tensor("wo", [NHL * HS, H], bf16, kind="ExternalInput").ap()
    bq_d = nc.dram_tensor("bq", [128, NHP], fp32, kind="ExternalInput").ap()
    bk_d = nc.dram_tensor("bk", [128, NHP], fp32, kind="ExternalInput").ap()
    bv_d = nc.dram_tensor("bv", [128, NHL * HS], fp32, kind="ExternalInput").ap()
    mask_d = nc.dram_tensor("mask", [128, SKT], fp32, kind="ExternalInput").ap()
    out_d = nc.dram_tensor("out", [S, H], fp32, kind="ExternalOutput").ap()

    with tile.TileContext(nc) as tc:
        with (
            tc.tile_pool(name="const", bufs=1) as const,
            tc.tile_pool(name="xt", bufs=1) as xtp,
            tc.tile_pool(name="vsb", bufs=1) as vsb,
            tc.tile_pool(name="qkt", bufs=2) as qkt,
            tc.tile_pool(name="combt", bufs=1) as combtp,
            tc.tile_pool(name="attn", bufs=4) as attnp,
            tc.tile_pool(name="small", bufs=4) as smallp,
            tc.tile_pool(name="outsb", bufs=3) as outsbp,
            tc.tile_pool(name="ps_sc", bufs=2, space="PSUM") as ps_sc,
            tc.tile_pool(name="ps_cb", bufs=2, space="PSUM") as ps_cb,
            tc.tile_pool(name="ps_pj", bufs=2, space="PSUM") as ps_pj,
        ):
            # ---- constants / weights into SBUF ----
            wq_sb = []
            wk_sb = []
            for c in range(HCHUNKS):
                tq = const.tile([128, NHL * HS], bf16, tag=f"wq{c}")
                nc.sync.dma_start(tq[:], wq_d[c * 128:(c + 1) * 128, :])
                wq_sb.append(tq)
                tk = const.tile([128, NHL * HS], bf16, tag=f"wk{c}")
                nc.sync.dma_start(tk[:], wk_d[c * 128:(c + 1) * 128, :])
                wk_sb.append(tk)
            wv_sb = []
            for c in range(HCHUNKS):
                tv = const.tile([128, NHL * HS], bf16, tag=f"wv{c}")
                nc.sync.dma_start(tv[:], wv_d[c * 128:(c + 1) * 128, :])
                wv_sb.append(tv)
            # Wo as 6 chunks of 64 rows (K=64 contraction in out-proj)
            wo_sb = []
            for c in range(NHL):
                to = const.tile([64, H], bf16, tag=f"wo{c}")
                nc.sync.dma_start(to[:], wo_d[c * 64:(c + 1) * 64, :])
                wo_sb.append(to)
            bq_sb = const.tile([128, NHP], fp32, tag="bq")
            nc.sync.dma_start(bq_sb[:], bq_d[:])
            bk_sb = const.tile([128, NHP], fp32, tag="bk")
            nc.sync.dma_start(bk_sb[:], bk_d[:])
            bv_sb = const.tile([128, NHL * HS], fp32, tag="bv")
            nc.sync.dma_start(bv_sb[:], bv_d[:])
            mask_sb = const.tile([128, SKT], fp32, tag="mask")
            nc.sync.dma_start(mask_sb[:], mask_d[:])

            # ---- x^T via DMA transpose ----
            xt = []
            for c in range(HCHUNKS):
                t = xtp.tile([128, S], bf16, tag=f"xt{c}")
                nc.sync.dma_start_transpose(t[:], x_d[:, c * 128:(c + 1) * 128])
                xt.append(t)

            # ---- V projection: V[s, h*65+d], col h*65+64 = 1.0 ----
            v_sb = []
            for kt in range(SKT):
                vt = vsb.tile([128, NHL, HS + 1], bf16, tag=f"v{kt}")
                ps = ps_pj.tile([128, 512], fp32, tag="pj")
                for c in range(HCHUNKS):
                    nc.tensor.matmul(
                        ps[:, :NHL * HS],
                        lhsT=xt[c][:, kt * 128:(kt + 1) * 128],
                        rhs=wv_sb[c][:],
                        start=(c == 0),
                        stop=(c == HCHUNKS - 1),
                    )
                nc.vector.tensor_add(
                    vt[:, :, 0:HS],
                    ps[:, :NHL * HS].rearrange("p (h d) -> p h d", h=NHL),
                    bv_sb[:].rearrange("p (h d) -> p h d", h=NHL),
                )
                nc.vector.memset(vt[:, :, HS:HS + 1], 1.0)
                v_sb.append(vt)

            combt_a = []
            combt_b = []
            for hp in range(NHP):
                combt_a.append(combtp.tile([64, S], bf16, tag=f"cta{hp}", name=f"cta{hp}"))
                combt_b.append(combtp.tile([64, S], bf16, tag=f"ctb{hp}", name=f"ctb{hp}"))

            for hp in range(NHP):
                # ---- Q^T, K^T projections for this head pair ----
                qt_t = qkt.tile([128, S], bf16, tag="qt")
                kt_t = qkt.tile([128, S], bf16, tag="kt")
                for dst, w_sb, b_sb in ((qt_t, wq_sb, bq_sb), (kt_t, wk_sb, bk_sb)):
                    for sq in range(SQT):
                        ps = ps_pj.tile([128, 512], fp32, tag="pj")
                        for c in range(HCHUNKS):
                            nc.tensor.matmul(
                                ps[:],
                                lhsT=w_sb[c][:, hp * 128:(hp + 1) * 128],
                                rhs=xt[c][:, sq * QW:(sq + 1) * QW],
                                start=(c == 0),
                                stop=(c == HCHUNKS - 1),
                            )
                        nc.vector.tensor_scalar_add(
                            dst[:, sq * QW:(sq + 1) * QW], ps[:],
                            b_sb[:, hp:hp + 1],
                        )

                # ---- attention ----
                for sq in range(SQT):
                    cb_a = ps_cb.tile([65, 512], fp32, tag="cb")
                    cb_b = ps_cb.tile([65, 512], fp32, tag="cb")
                    for kt in range(SKT):
                        sc = ps_sc.tile([128, 1024], fp32, tag="sc")
                        # two heads packed via PE row tiling (K=64 each)
                        nc.tensor.matmul(
                            sc[:, 0:512],
                            lhsT=kt_t[0:64, kt * 128:(kt + 1) * 128],
                            rhs=qt_t[0:64, sq * QW:(sq + 1) * QW],
                            start=True, stop=True,
                        )
                        nc.tensor.matmul(
                            sc[:, 512:1024],
                            lhsT=kt_t[64:128, kt * 128:(kt + 1) * 128],
                            rhs=qt_t[64:128, sq * QW:(sq + 1) * QW],
                            start=True, stop=True,
                        )
                        at = attnp.tile([128, 1024], bf16, tag="at")
                        nc.scalar.activation(
                            at[:], sc[:], AF.Exp,
                            bias=mask_sb[:, kt:kt + 1], scale=0.125,
                        )
                        nc.tensor.matmul(
                            cb_a[:],
                            lhsT=v_sb[kt][:, 2 * hp, :],
                            rhs=at[:, 0:512],
                            start=(kt == 0), stop=(kt == SKT - 1),
                        )
                        nc.tensor.matmul(
                            cb_b[:],
                            lhsT=v_sb[kt][:, 2 * hp + 1, :],
                            rhs=at[:, 512:1024],
                            start=(kt == 0), stop=(kt == SKT - 1),
                        )
                    # normalize: comb rows 0..63 / denom(row 64)
                    for cb, ct in ((cb_a, combt_a[hp]), (cb_b, combt_b[hp])):
                        rc = smallp.tile([65, 512], fp32, tag="rc")
                        nc.vector.reciprocal(rc[64:65, :], cb[64:65, :])
                        rc0 = smallp.tile([1, 512], fp32, tag="rc0")
                        nc.sync.dma_start(rc0[:], rc[64:65, :])
                        bc = smallp.tile([64, 512], fp32, tag="bc")
                        nc.gpsimd.partition_broadcast(bc[:], rc0[:])
                        nc.vector.tensor_mul(
                            ct[:, sq * QW:(sq + 1) * QW], cb[0:64, :], bc[:],
                        )

            # ---- output projection (partial; host adds the two halves + bo)
            for st in range(S // 128):
                ps0 = ps_pj.tile([128, 512], fp32, tag="pj")
                ps1 = ps_pj.tile([128, 512], fp32, tag="pj")
                for half, ps in ((0, ps0), (1, ps1)):
                    for c in range(NHL):
                        ct = combt_a[c // 2] if c % 2 == 0 else combt_b[c // 2]
                        nc.tensor.matmul(
                            ps[:, 0:384],
                            lhsT=ct[:, st * 128:(st + 1) * 128],
                            rhs=wo_sb[c][:, half * 384:(half + 1) * 384],
                            start=(c == 0), stop=(c == NHL - 1),
                        )
                ot = outsbp.tile([128, H], fp32, tag="ot")
                nc.vector.tensor_copy(ot[:, 0:384], ps0[:, 0:384])
                nc.vector.tensor_copy(ot[:, 384:768], ps1[:, 0:384])
                nc.sync.dma_start(out_d[st * 128:(st + 1) * 128, :], ot[:])

    nc.compile()
    return nc


def _get_compiled():
    global _COMPILED
    if _COMPILED is None:
        _COMPILED = _build()
    return _COMPILED


def _prep_core_inputs(x, mask, Wq, bq, Wk, bk, Wv, bv, Wo, core):
    b, hg = core // 2, core % 2
    lo, hi = hg * NHL * HS, (hg + 1) * NHL * HS
    bf = ml_dtypes.bfloat16
    return {
        "x": np.ascontiguousarray(x[b]).astype(bf),
        "wq": np.ascontiguousarray(Wq[:, lo:hi]).astype(bf),
        "wk": np.ascontiguousarray(Wk[:, lo:hi]).astype(bf),
        "wv": np.ascontiguousarray(Wv[:, lo:hi]).astype(bf),
        "wo": np.ascontiguousarray(Wo[lo:hi, :]).astype(bf),
        "bq": np.ascontiguousarray(bq[lo:hi].reshape(NHP, 128).T).astype(np.float32),
        "bk": np.ascontiguousarray(bk[lo:hi].reshape(NHP, 128).T).astype(np.float32),
        "bv": np.tile(bv[lo:hi][None, :], (128, 1)).astype(np.float32),
        "mask": np.ascontiguousarray(
            mask[b, 0, 0].reshape(SKT, 128).T).astype(np.float32),
    }


def kernel(x, additive_attention_mask, Wq, bq, Wk, bk, Wv, bv, Wo, bo):
    from concourse import bass2jax

    x = np.asarray(x, dtype=np.float32)
    mask = np.asarray(additive_attention_mask, dtype=np.float32)
    args = [np.asarray(a, dtype=np.float32) for a in (Wq, bq, Wk, bk, Wv, bv, Wo)]
    Wq, bq, Wk, bk, Wv, bv, Wo = args
    bo = np.asarray(bo, dtype=np.float32)

    nc = _get_compiled()
    in_maps = [
        _prep_core_inputs(x, mask, Wq, bq, Wk, bk, Wv, bv, Wo, c)
        for c in range(N_CORES)
    ]
    results = bass2jax.run_bass_via_pjrt(nc, in_maps, n_cores=N_CORES)

    out = np.empty((B, S, H), dtype=np.float32)
    for b in range(B):
        out[b] = results[2 * b]["out"] + results[2 * b + 1]["out"] + bo
    return out


# revision 4
# speedup vs baseline: 1.2679x; 1.2679x over previous
"""BERT self-attention (B=4, S=2048, H=768, 12 heads x 64) on 8 trn2 cores.

Sharding: core c = batch (c//2) x head-half (c%2, 6 heads each).
Each core computes Q/K/V projections for its 6 heads, attention, and a
partial output projection (its heads' slice of Wo). Host sums the two
partials per batch and adds bo.

On-device layout (per core):
  xT   [768, 2048]  bf16  (DMA-transposed x)
  QT/KT per head-pair [128=2x64, 2048] bf16   (head-dim on partitions)
  V    16 tiles [128 keys, 6 heads x 65] bf16 (col 64 of each head = 1.0)
  scores^T [128 keys, 2x512 q] fp32 PSUM (two heads packed via row tiling)
  exp on ScalarE (scale=1/8, bias=mask column), out bf16
  attn@V -> comb [65, 512] PSUM; row 64 = softmax denominator
  combT_A/B [64, 2048] bf16, scaled by 1/denom
  out-proj: 6 x K=64 chunks -> psum [128, 384] x2 -> out [2048, 768] fp32
"""

import numpy as np
import ml_dtypes

B, S, H = 4, 2048, 768
NH, HS = 12, 64
NHL = 6              # heads per core
NHP = 3              # head pairs per core
HCHUNKS = 6          # 768 / 128 contraction chunks
SKT = 16             # key tiles of 128
SQT = 4              # query tiles of 512
QW = 512             # query tile width
N_CORES = 8

_COMPILED = None


def _build():
    import concourse.bass as bass
    import concourse.mybir as mybir
    import concourse.tile as tile
    from concourse import bacc

    fp32 = mybir.dt.float32
    bf16 = mybir.dt.bfloat16
    AF = mybir.ActivationFunctionType

    nc = bacc.Bacc("TRN2", target_bir_lowering=False, debug=False)

    x_d = nc.dram_tensor("x", [S, H], bf16, kind="ExternalInput").ap()
    wq_d = nc.dram_tensor("wq", [H, NHL * HS], bf16, kind="ExternalInput").ap()
    wk_d = nc.dram_tensor("wk", [H, NHL * HS], bf16, kind="ExternalInput").ap()
    wv_d = nc.dram_tensor("wv", [H, NHL * HS], bf16, kind="ExternalInput").ap()
    wo_d = nc.dram_tensor("wo", [NHL * HS, H], bf16, kind="ExternalInput").ap()
    bq_d = nc.dram_tensor("bq", [128, NHP], fp32, kind="ExternalInput").ap()
    bk_d = nc.dram_tensor("bk", [128, NHP], fp32, kind="ExternalInput").ap()
    bv_d = nc.dram_tensor("bv", [128, NHL * HS], fp32, kind="ExternalInput").ap()
    mask_d = nc.dram_tensor("mask", [128, SKT], fp32, kind="ExternalInput").ap()
    out_d = nc.dram_tensor("out", [S, H], fp32, kind="ExternalOutput").ap()

    with tile.TileContext(nc) as tc:
        with (
            tc.tile_pool(name="const", bufs=1) as const,
            tc.tile_pool(name="xt", bufs=1) as xtp,
            tc.tile_pool(name="vsb", bufs=1) as vsb,
            tc.tile_pool(name="qkt", bufs=2) as qkt,
            tc.tile_pool(name="combt", bufs=1) as combtp,
            tc.tile_pool(name="attn", bufs=4) as attnp,
            tc.tile_pool(name="small", bufs=4) as smallp,
            tc.tile_pool(name="outsb", bufs=3) as outsbp,
            tc.tile_pool(name="ps_sc", bufs=2, space="PSUM") as ps_sc,
            tc.tile_pool(name="ps_cb", bufs=2, space="PSUM") as ps_cb,
            tc.tile_pool(name="ps_pj", bufs=2, space="PSUM") as ps_pj,
        ):
            # ---- constants / weights into SBUF ----
            wq_sb = []
            wk_sb = []
            for c in range(HCHUNKS):
                tq = const.tile([128, NHL * HS], bf16, tag=f"wq{c}")
                nc.sync.dma_start(tq[:], wq_d[c * 128:(c + 1) * 128, :])
                wq_sb.append(tq)
                tk = const.tile([128, NHL * HS], bf16, tag=f"wk{c}")
                nc.sync.dma_start(tk[:], wk_d[c * 128:(c + 1) * 128, :])
                wk_sb.append(tk)
            wv_sb = []
            for c in range(HCHUNKS):
                tv = const.tile([128, NHL * HS], bf16, tag=f"wv{c}")
                nc.sync.dma_start(tv[:], wv_d[c * 128:(c + 1) * 128, :])
                wv_sb.append(tv)
            # Wo as 6 chunks of 64 rows (K=64 contraction in out-proj)
            wo_sb = []
            for c in range(NHL):
                to = const.tile([64, H], bf16, tag=f"wo{c}")
                nc.sync.dma_start(to[:], wo_d[c * 64:(c + 1) * 64, :])
                wo_sb.append(to)
            bq_sb = const.tile([128, NHP], fp32, tag="bq")
            nc.sync.dma_start(bq_sb[:], bq_d[:])
            bk_sb = const.tile([128, NHP], fp32, tag="bk")
            nc.sync.dma_start(bk_sb[:], bk_d[:])
            bv_sb = const.tile([128, NHL * HS], fp32, tag="bv")
            nc.sync.dma_start(bv_sb[:], bv_d[:])
            mask_sb = const.tile([128, SKT], fp32, tag="mask")
            nc.sync.dma_start(mask_sb[:], mask_d[:])

            # ---- x^T via DMA transpose ----
            xt = []
            for c in range(HCHUNKS):
                t = xtp.tile([128, S], bf16, tag=f"xt{c}")
                nc.sync.dma_start_transpose(t[:], x_d[:, c * 128:(c + 1) * 128])
                xt.append(t)

            # ---- V projection: V[s, h*65+d], col h*65+64 = 1.0 ----
            v_sb = []
            for kt in range(SKT):
                vt = vsb.tile([128, NHL, HS + 1], bf16, tag=f"v{kt}")
                ps = ps_pj.tile([128, 512], fp32, tag="pj")
                for c in range(HCHUNKS):
                    nc.tensor.matmul(
                        ps[:, :NHL * HS],
                        lhsT=xt[c][:, kt * 128:(kt + 1) * 128],
                        rhs=wv_sb[c][:],
                        start=(c == 0),
                        stop=(c == HCHUNKS - 1),
                    )
                nc.vector.tensor_add(
                    vt[:, :, 0:HS],
                    ps[:, :NHL * HS].rearrange("p (h d) -> p h d", h=NHL),
                    bv_sb[:].rearrange("p (h d) -> p h d", h=NHL),
                )
                nc.vector.memset(vt[:, :, HS:HS + 1], 1.0)
                v_sb.append(vt)

            combt_a = []
            combt_b = []
            for hp in range(NHP):
                combt_a.append(combtp.tile([64, S], bf16, tag=f"cta{hp}", name=f"cta{hp}"))
                combt_b.append(combtp.tile([64, S], bf16, tag=f"ctb{hp}", name=f"ctb{hp}"))

            for hp in range(NHP):
                # ---- Q^T, K^T projections for this head pair ----
                qt_t = qkt.tile([128, S], bf16, tag="qt")
                kt_t = qkt.tile([128, S], bf16, tag="kt")
                for dst, w_sb, b_sb in ((qt_t, wq_sb, bq_sb), (kt_t, wk_sb, bk_sb)):
                    for sq in range(SQT):
                        ps = ps_pj.tile([128, 512], fp32, tag="pj")
                        for c in range(HCHUNKS):
                            nc.tensor.matmul(
                                ps[:],
                                lhsT=w_sb[c][:, hp * 128:(hp + 1) * 128],
                                rhs=xt[c][:, sq * QW:(sq + 1) * QW],
                                start=(c == 0),
                                stop=(c == HCHUNKS - 1),
                            )
                        nc.vector.tensor_scalar_add(
                            dst[:, sq * QW:(sq + 1) * QW], ps[:],
                            b_sb[:, hp:hp + 1],
                        )

                # ---- attention (software-pipelined: scores kt+1 issue
                # before attn@V kt so the in-order PE never queues behind
                # the exp it is waiting for) ----
                for sq in range(SQT):
                    cb_a = ps_cb.tile([65, 512], fp32, tag="cb")
                    cb_b = ps_cb.tile([65, 512], fp32, tag="cb")

                    def scores(kt):
                        sc = ps_sc.tile([128, 1024], fp32, tag="sc", name="sc")
                        # two heads packed via PE row tiling (K=64 each)
                        nc.tensor.matmul(
                            sc[:, 0:512],
                            lhsT=kt_t[0:64, kt * 128:(kt + 1) * 128],
                            rhs=qt_t[0:64, sq * QW:(sq + 1) * QW],
                            start=True, stop=True,
                        )
                        nc.tensor.matmul(
                            sc[:, 512:1024],
                            lhsT=kt_t[64:128, kt * 128:(kt + 1) * 128],
                            rhs=qt_t[64:128, sq * QW:(sq + 1) * QW],
                            start=True, stop=True,
                        )
                        return sc

                    sc_cur = scores(0)
                    for kt in range(SKT):
                        sc_nxt = scores(kt + 1) if kt + 1 < SKT else None
                        at = attnp.tile([128, 1024], bf16, tag="at")
                        nc.scalar.activation(
                            at[:], sc_cur[:], AF.Exp,
                            bias=mask_sb[:, kt:kt + 1], scale=0.125,
                        )
                        nc.tensor.matmul(
                            cb_a[:],
                            lhsT=v_sb[kt][:, 2 * hp, :],
                            rhs=at[:, 0:512],
                            start=(kt == 0), stop=(kt == SKT - 1),
                        )
                        nc.tensor.matmul(
                            cb_b[:],
                            lhsT=v_sb[kt][:, 2 * hp + 1, :],
                            rhs=at[:, 512:1024],
                            start=(kt == 0), stop=(kt == SKT - 1),
                        )
                        sc_cur = sc_nxt
                    # normalize: comb rows 0..63 / denom(row 64).
                    # One copy to SBUF frees the PSUM bank early.
                    for cb, ct in ((cb_a, combt_a[hp]), (cb_b, combt_b[hp])):
                        cbs = smallp.tile([65, 512], fp32, tag="cbs")
                        nc.vector.tensor_copy(cbs[:], cb[:])
                        rc0 = smallp.tile([1, 512], fp32, tag="rc0")
                        nc.sync.dma_start(rc0[:], cbs[64:65, :])
                        rc1 = smallp.tile([1, 512], fp32, tag="rc1")
                        # approx recip is partition-0 only on HW
                        nc.vector.reciprocal_approx_fast(rc1[:], rc0[:])
                        bc = smallp.tile([64, 512], fp32, tag="bc")
                        nc.gpsimd.partition_broadcast(bc[:], rc1[:])
                        nc.vector.tensor_mul(
                            ct[:, sq * QW:(sq + 1) * QW], cbs[0:64, :], bc[:],
                        )

            # ---- output projection (partial; host adds the two halves + bo)
            for st in range(S // 128):
                ps0 = ps_pj.tile([128, 512], fp32, tag="pj")
                ps1 = ps_pj.tile([128, 512], fp32, tag="pj")
                for half, ps in ((0, ps0), (1, ps1)):
                    for c in range(NHL):
                        ct = combt_a[c // 2] if c % 2 == 0 else combt_b[c // 2]
                        nc.tensor.matmul(
                            ps[:, 0:384],
                            lhsT=ct[:, st * 128:(st + 1) * 128],
                            rhs=wo_sb[c][:, half * 384:(half + 1) * 384],
                            start=(c == 0), stop=(c == NHL - 1),
                        )
                ot = outsbp.tile([128, H], fp32, tag="ot")
                nc.vector.tensor_copy(ot[:, 0:384], ps0[:, 0:384])
                nc.vector.tensor_copy(ot[:, 384:768], ps1[:, 0:384])
                nc.sync.dma_start(out_d[st * 128:(st + 1) * 128, :], ot[:])

    nc.compile()
    return nc


def _get_compiled():
    global _COMPILED
    if _COMPILED is None:
        _COMPILED = _build()
    return _COMPILED


def _prep_core_inputs(x, mask, Wq, bq, Wk, bk, Wv, bv, Wo, core):
    b, hg = core // 2, core % 2
    lo, hi = hg * NHL * HS, (hg + 1) * NHL * HS
    bf = ml_dtypes.bfloat16
    return {
        "x": np.ascontiguousarray(x[b]).astype(bf),
        "wq": np.ascontiguousarray(Wq[:, lo:hi]).astype(bf),
        "wk": np.ascontiguousarray(Wk[:, lo:hi]).astype(bf),
        "wv": np.ascontiguousarray(Wv[:, lo:hi]).astype(bf),
        "wo": np.ascontiguousarray(Wo[lo:hi, :]).astype(bf),
        "bq": np.ascontiguousarray(bq[lo:hi].reshape(NHP, 128).T).astype(np.float32),
        "bk": np.ascontiguousarray(bk[lo:hi].reshape(NHP, 128).T).astype(np.float32),
        "bv": np.tile(bv[lo:hi][None, :], (128, 1)).astype(np.float32),
        "mask": np.ascontiguousarray(
            mask[b, 0, 0].reshape(SKT, 128).T).astype(np.float32),
    }


def kernel(x, additive_attention_mask, Wq, bq, Wk, bk, Wv, bv, Wo, bo):
    from concourse import bass2jax

    x = np.asarray(x, dtype=np.float32)
    mask = np.asarray(additive_attention_mask, dtype=np.float32)
    args = [np.asarray(a, dtype=np.float32) for a in (Wq, bq, Wk, bk, Wv, bv, Wo)]
    Wq, bq, Wk, bk, Wv, bv, Wo = args
    bo = np.asarray(bo, dtype=np.float32)

    nc = _get_compiled()
    in_maps = [
        _prep_core_inputs(x, mask, Wq, bq, Wk, bk, Wv, bv, Wo, c)
        for c in range(N_CORES)
    ]
    results = bass2jax.run_bass_via_pjrt(nc, in_maps, n_cores=N_CORES)

    out = np.empty((B, S, H), dtype=np.float32)
    for b in range(B):
        out[b] = results[2 * b]["out"] + results[2 * b + 1]["out"] + bo
    return out


# revision 5
# speedup vs baseline: 1.3264x; 1.0462x over previous
"""BERT self-attention (B=4, S=2048, H=768, 12 heads x 64) on 8 trn2 cores.

Sharding: core c = batch (c//2) x head-half (c%2, 6 heads each).
Each core computes Q/K/V projections for its 6 heads, attention, and a
partial output projection (its heads' slice of Wo). Host sums the two
partials per batch and adds bo.

On-device layout (per core):
  xT   [768, 2048]  bf16  (DMA-transposed x)
  QT/KT per head-pair [128=2x64, 2048] bf16   (head-dim on partitions)
  V    16 tiles [128 keys, 6 heads x 65] bf16 (col 64 of each head = 1.0)
  scores^T [128 keys, 2x512 q] fp32 PSUM (two heads packed via row tiling)
  exp on ScalarE (scale=1/8, bias=mask column), out bf16
  attn@V -> comb [65, 512] PSUM; row 64 = softmax denominator
  combT_A/B [64, 2048] bf16, scaled by 1/denom
  out-proj: 6 x K=64 chunks accumulated in SBUF out_acc, fp32

The attention loop is ACT(exp)-bound; Q/K projections of the next head
pair and the output projection of previous head pairs are interleaved
into the PE bubbles of the attention loop (in-order PE: the injected
matmuls sit between scores(kt+1) and attn@V(kt) in program order).
"""

import numpy as np
import ml_dtypes

B, S, H = 4, 2048, 768
NH, HS = 12, 64
NHL = 6              # heads per core
NHP = 3              # head pairs per core
HCHUNKS = 6          # 768 / 128 contraction chunks
SKT = 16             # key tiles of 128
SQT = 4              # query tiles of 512
QW = 512             # query tile width
N_CORES = 8

_COMPILED = None


def _build():
    import concourse.bass as bass
    import concourse.mybir as mybir
    import concourse.tile as tile
    from concourse import bacc

    fp32 = mybir.dt.float32
    bf16 = mybir.dt.bfloat16
    AF = mybir.ActivationFunctionType

    nc = bacc.Bacc("TRN2", target_bir_lowering=False, debug=False)

    x_d = nc.dram_tensor("x", [S, H], bf16, kind="ExternalInput").ap()
    wq_d = nc.dram_tensor("wq", [H, NHL * HS], bf16, kind="ExternalInput").ap()
    wk_d = nc.dram_tensor("wk", [H, NHL * HS], bf16, kind="ExternalInput").ap()
    wv_d = nc.dram_tensor("wv", [H, NHL * HS], bf16, kind="ExternalInput").ap()
    wo_d = nc.dram_tensor("wo", [NHL * HS, H], bf16, kind="ExternalInput").ap()
    bq_d = nc.dram_tensor("bq", [128, NHP], fp32, kind="ExternalInput").ap()
    bk_d = nc.dram_tensor("bk", [128, NHP], fp32, kind="ExternalInput").ap()
    bv_d = nc.dram_tensor("bv", [128, NHL * HS], fp32, kind="ExternalInput").ap()
    mask_d = nc.dram_tensor("mask", [128, SKT], fp32, kind="ExternalInput").ap()
    out_d = nc.dram_tensor("out", [S, H], fp32, kind="ExternalOutput").ap()

    with tile.TileContext(nc) as tc:
        with (
            tc.tile_pool(name="const", bufs=1) as const,
            tc.tile_pool(name="xt", bufs=1) as xtp,
            tc.tile_pool(name="vsb", bufs=1) as vsb,
            tc.tile_pool(name="qkt", bufs=2) as qkt,
            tc.tile_pool(name="combt", bufs=1) as combtp,
            tc.tile_pool(name="oacc", bufs=1) as oaccp,
            tc.tile_pool(name="attn", bufs=4) as attnp,
            tc.tile_pool(name="small", bufs=4) as smallp,
            tc.tile_pool(name="ps_sc", bufs=2, space="PSUM") as ps_sc,
            tc.tile_pool(name="ps_cb", bufs=2, space="PSUM") as ps_cb,
            tc.tile_pool(name="ps_pj", bufs=2, space="PSUM") as ps_pj,
        ):
            # ---- x^T first (V-proj critical path), then weights ----
            xt = []
            for c in range(HCHUNKS):
                t = xtp.tile([128, S], bf16, tag=f"xt{c}", name=f"xt{c}")
                nc.sync.dma_start_transpose(t[:], x_d[:, c * 128:(c + 1) * 128])
                xt.append(t)
            wv_sb = []
            for c in range(HCHUNKS):
                tv = const.tile([128, NHL * HS], bf16, tag=f"wv{c}", name=f"wv{c}")
                nc.sync.dma_start(tv[:], wv_d[c * 128:(c + 1) * 128, :])
                wv_sb.append(tv)
            wq_sb = []
            wk_sb = []
            for c in range(HCHUNKS):
                tq = const.tile([128, NHL * HS], bf16, tag=f"wq{c}", name=f"wq{c}")
                nc.sync.dma_start(tq[:], wq_d[c * 128:(c + 1) * 128, :])
                wq_sb.append(tq)
                tk = const.tile([128, NHL * HS], bf16, tag=f"wk{c}", name=f"wk{c}")
                nc.sync.dma_start(tk[:], wk_d[c * 128:(c + 1) * 128, :])
                wk_sb.append(tk)
            bq_sb = const.tile([128, NHP], fp32, tag="bq")
            nc.sync.dma_start(bq_sb[:], bq_d[:])
            bk_sb = const.tile([128, NHP], fp32, tag="bk")
            nc.sync.dma_start(bk_sb[:], bk_d[:])
            bv_sb = const.tile([128, NHL * HS], fp32, tag="bv")
            nc.sync.dma_start(bv_sb[:], bv_d[:])
            mask_sb = const.tile([128, SKT], fp32, tag="mask")
            nc.sync.dma_start(mask_sb[:], mask_d[:])
            # Wo as 6 chunks of 64 rows (K=64 contraction in out-proj)
            wo_sb = []
            for c in range(NHL):
                to = const.tile([64, H], bf16, tag=f"wo{c}", name=f"wo{c}")
                nc.sync.dma_start(to[:], wo_d[c * 64:(c + 1) * 64, :])
                wo_sb.append(to)

            # ---- V projection: V[s, h*65+d], col h*65+64 = 1.0 ----
            v_sb = []
            for kt in range(SKT):
                vt = vsb.tile([128, NHL, HS + 1], bf16, tag=f"v{kt}", name=f"v{kt}")
                ps = ps_pj.tile([128, 512], fp32, tag="pj", name="psv")
                for c in range(HCHUNKS):
                    nc.tensor.matmul(
                        ps[:, :NHL * HS],
                        lhsT=xt[c][:, kt * 128:(kt + 1) * 128],
                        rhs=wv_sb[c][:],
                        start=(c == 0),
                        stop=(c == HCHUNKS - 1),
                    )
                nc.vector.tensor_add(
                    vt[:, :, 0:HS],
                    ps[:, :NHL * HS].rearrange("p (h d) -> p h d", h=NHL),
                    bv_sb[:].rearrange("p (h d) -> p h d", h=NHL),
                )
                nc.vector.memset(vt[:, :, HS:HS + 1], 1.0)
                v_sb.append(vt)

            combt_a = []
            combt_b = []
            for hp in range(NHP):
                combt_a.append(combtp.tile([64, S], bf16, tag=f"cta{hp}", name=f"cta{hp}"))
                combt_b.append(combtp.tile([64, S], bf16, tag=f"ctb{hp}", name=f"ctb{hp}"))
            # partial output accumulator [128, st, 768]
            out_acc = oaccp.tile([128, S // 128, H], fp32, tag="oacc")

            def emit_qkt(hp):
                """Q^T/K^T projection for head pair hp; returns (qt, kt, units).
                Each unit emits one matmul (plus drain on the last)."""
                qt_t = qkt.tile([128, S], bf16, tag="qt", name=f"qt{hp}")
                kt_t = qkt.tile([128, S], bf16, tag="kt", name=f"kt{hp}")
                units = []
                for dst, w_sb, b_sb in ((qt_t, wq_sb, bq_sb), (kt_t, wk_sb, bk_sb)):
                    for sq in range(SQT):
                        st8 = {}

                        def unit(c, dst=dst, w_sb=w_sb, b_sb=b_sb, sq=sq, st8=st8):
                            if c == 0:
                                st8["ps"] = ps_pj.tile(
                                    [128, 512], fp32, tag="pj", name="psq")
                            nc.tensor.matmul(
                                st8["ps"][:],
                                lhsT=w_sb[c][:, hp * 128:(hp + 1) * 128],
                                rhs=xt[c][:, sq * QW:(sq + 1) * QW],
                                start=(c == 0),
                                stop=(c == HCHUNKS - 1),
                            )
                            if c == HCHUNKS - 1:
                                nc.vector.tensor_scalar_add(
                                    dst[:, sq * QW:(sq + 1) * QW], st8["ps"][:],
                                    b_sb[:, hp:hp + 1],
                                )

                        for c in range(HCHUNKS):
                            units.append(lambda c=c, u=unit: u(c))
                return qt_t, kt_t, units

            def emit_outproj_unit(hp, st, half, phase, st8, stream_out=False):
                """One of two matmuls of out-proj psum chain (st, half) for
                head pair hp; phase 0 = first chunk, 1 = second + drain."""
                if phase == 0:
                    st8["ps"] = ps_pj.tile([128, 512], fp32, tag="pj", name="pso")
                ct = combt_a[hp] if phase == 0 else combt_b[hp]
                nc.tensor.matmul(
                    st8["ps"][:, 0:384],
                    lhsT=ct[:, st * 128:(st + 1) * 128],
                    rhs=wo_sb[2 * hp + phase][:, half * 384:(half + 1) * 384],
                    start=(phase == 0), stop=(phase == 1),
                )
                if phase == 1:
                    dst = out_acc[:, st, half * 384:(half + 1) * 384]
                    if hp == 0:
                        nc.vector.tensor_copy(dst, st8["ps"][:, 0:384])
                    else:
                        nc.vector.tensor_add(dst, dst, st8["ps"][:, 0:384])
                    if stream_out:
                        nc.sync.dma_start(
                            out_d[st * 128:(st + 1) * 128, :], out_acc[:, st, :])

            def outproj_units(hp):
                units = []
                for st in range(S // 128):
                    for half in range(2):
                        st8 = {}
                        for phase in range(2):
                            units.append(
                                lambda hp=hp, st=st, half=half, phase=phase,
                                st8=st8: emit_outproj_unit(hp, st, half, phase, st8))
                return units

            qt_t, kt_t, units0 = emit_qkt(0)
            for u in units0:
                u()

            for hp in range(NHP):
                # work to interleave into this head pair's attention
                inject = []
                if hp + 1 < NHP:
                    nqt, nkt, inject = emit_qkt(hp + 1)
                if hp == 1:
                    inject = inject + outproj_units(0)
                elif hp == 2:
                    inject = outproj_units(1)
                inj_i = 0

                for sq in range(SQT):
                    cb_a = ps_cb.tile([65, 512], fp32, tag="cb", name="cba")
                    cb_b = ps_cb.tile([65, 512], fp32, tag="cb", name="cbb")

                    def scores(kt):
                        sc = ps_sc.tile([128, 1024], fp32, tag="sc", name="sc")
                        nc.tensor.matmul(
                            sc[:, 0:512],
                            lhsT=kt_t[0:64, kt * 128:(kt + 1) * 128],
                            rhs=qt_t[0:64, sq * QW:(sq + 1) * QW],
                            start=True, stop=True,
                        )
                        nc.tensor.matmul(
                            sc[:, 512:1024],
                            lhsT=kt_t[64:128, kt * 128:(kt + 1) * 128],
                            rhs=qt_t[64:128, sq * QW:(sq + 1) * QW],
                            start=True, stop=True,
                        )
                        return sc

                    sc_cur = scores(0)
                    for kt in range(SKT):
                        sc_nxt = scores(kt + 1) if kt + 1 < SKT else None
                        at = attnp.tile([128, 1024], bf16, tag="at")
                        nc.scalar.activation(
                            at[:], sc_cur[:], AF.Exp,
                            bias=mask_sb[:, kt:kt + 1], scale=0.125,
                        )
                        # fill the PE exp-wait bubble with independent work
                        for _ in range(2):
                            if inj_i < len(inject):
                                inject[inj_i]()
                                inj_i += 1
                        nc.tensor.matmul(
                            cb_a[:],
                            lhsT=v_sb[kt][:, 2 * hp, :],
                            rhs=at[:, 0:512],
                            start=(kt == 0), stop=(kt == SKT - 1),
                        )
                        nc.tensor.matmul(
                            cb_b[:],
                            lhsT=v_sb[kt][:, 2 * hp + 1, :],
                            rhs=at[:, 512:1024],
                            start=(kt == 0), stop=(kt == SKT - 1),
                        )
                        sc_cur = sc_nxt
                    # normalize: comb rows 0..63 / denom(row 64).
                    # One copy to SBUF frees the PSUM bank early.
                    for cb, ct in ((cb_a, combt_a[hp]), (cb_b, combt_b[hp])):
                        cbs = smallp.tile([65, 512], fp32, tag="cbs")
                        nc.vector.tensor_copy(cbs[:], cb[:])
                        rc0 = smallp.tile([1, 512], fp32, tag="rc0")
                        nc.sync.dma_start(rc0[:], cbs[64:65, :])
                        rc1 = smallp.tile([1, 512], fp32, tag="rc1")
                        # approx recip is partition-0 only on HW
                        nc.vector.reciprocal_approx_fast(rc1[:], rc0[:])
                        bc = smallp.tile([64, 512], fp32, tag="bc")
                        nc.gpsimd.partition_broadcast(bc[:], rc1[:])
                        nc.vector.tensor_mul(
                            ct[:, sq * QW:(sq + 1) * QW], cbs[0:64, :], bc[:],
                        )
                # leftover injected work
                while inj_i < len(inject):
                    inject[inj_i]()
                    inj_i += 1
                if hp + 1 < NHP:
                    qt_t, kt_t = nqt, nkt

            # ---- tail: out-proj of head pair 2, streaming the output ----
            for st in range(S // 128):
                for half in range(2):
                    st8 = {}
                    emit_outproj_unit(2, st, half, 0, st8)
                    emit_outproj_unit(2, st, half, 1, st8,
                                      stream_out=(half == 1))

    nc.compile()
    return nc


def _get_compiled():
    global _COMPILED
    if _COMPILED is None:
        _COMPILED = _build()
    return _COMPILED


def _prep_core_inputs(x, mask, Wq, bq, Wk, bk, Wv, bv, Wo, core):
    b, hg = core // 2, core % 2
    lo, hi = hg * NHL * HS, (hg + 1) * NHL * HS
    bf = ml_dtypes.bfloat16
    return {
        "x": np.ascontiguousarray(x[b]).astype(bf),
        "wq": np.ascontiguousarray(Wq[:, lo:hi]).astype(bf),
        "wk": np.ascontiguousarray(Wk[:, lo:hi]).astype(bf),
        "wv": np.ascontiguousarray(Wv[:, lo:hi]).astype(bf),
        "wo": np.ascontiguousarray(Wo[lo:hi, :]).astype(bf),
        "bq": np.ascontiguousarray(bq[lo:hi].reshape(NHP, 128).T).astype(np.float32),
        "bk": np.ascontiguousarray(bk[lo:hi].reshape(NHP, 128).T).astype(np.float32),
        "bv": np.tile(bv[lo:hi][None, :], (128, 1)).astype(np.float32),
        "mask": np.ascontiguousarray(
            mask[b, 0, 0].reshape(SKT, 128).T).astype(np.float32),
    }


def kernel(x, additive_attention_mask, Wq, bq, Wk, bk, Wv, bv, Wo, bo):
    from concourse import bass2jax

    x = np.asarray(x, dtype=np.float32)
    mask = np.asarray(additive_attention_mask, dtype=np.float32)
    args = [np.asarray(a, dtype=np.float32) for a in (Wq, bq, Wk, bk, Wv, bv, Wo)]
    Wq, bq, Wk, bk, Wv, bv, Wo = args
    bo = np.asarray(bo, dtype=np.float32)

    nc = _get_compiled()
    in_maps = [
        _prep_core_inputs(x, mask, Wq, bq, Wk, bk, Wv, bv, Wo, c)
        for c in range(N_CORES)
    ]
    results = bass2jax.run_bass_via_pjrt(nc, in_maps, n_cores=N_CORES)

    out = np.empty((B, S, H), dtype=np.float32)
    for b in range(B):
        out[b] = results[2 * b]["out"] + results[2 * b + 1]["out"] + bo
    return out
